# revision 30
# baseline (speedup 1.0000x reference)
"""CFRNet (moe_routing) Trainium2 Bass kernel.

Shared MLP 512->1024->1024->512 (ReLU,ReLU,linear+bias) followed by two
"risk net" experts 512->512->512->1 with per-sample binary treatment
routing.  Data-parallel across 8 NeuronCores: each core handles 8192 of
the 65536 samples; the (small) weights are replicated.

Layout strategy (per core):
  - Activations live feature-major in SBUF: [128 feat_part, ktile, samp].
    Then every layer is  out_fm = matmul(lhsT=W_tile, rhs=act_fm)  with the
    weights used in their natural [in_feat, out_feat] HBM layout and the
    output again feature-major.  The only transposes needed are at the
    input x (sample-major in HBM) and at the `out` tensor (sample-major
    output), done on the PE via transpose-mode matmuls.
  - All matmul operands are float32r (reduced-precision fp32): 1 PE pass,
    ~FP22 multiply precision, fp32 PSUM accumulate. float32 (4-pass) would
    be 4x slower; bf16 would be no faster than float32r here.
  - Per-sample treatment select runs on DVE rows [1, SB] per block.
"""

import numpy as np

import concourse.mybir as mybir
import concourse.tile as tile
from concourse import bacc
from concourse.bass_utils import run_bass_kernel_spmd

P = 128
N_CORES = 8
N_TOTAL = 65536
N_CORE = N_TOTAL // N_CORES  # 8192
D_IN, H1, H2, D_SH, HI = 512, 1024, 1024, 512, 512
SB = 512  # samples per block
F32R = mybir.dt.float32r
F32 = mybir.dt.float32
AF = mybir.ActivationFunctionType


def build_module(n_samp=N_CORE, repeat=1, dyn_repeat=1, experts=None,
                 skip_tp=False, dve_evac=False):
    """experts: per-block expert schedule, tuple of "A"/"C"/"both" of length
    n_samp//SB (None -> all "both").  Pure blocks run only one risk net —
    the host pre-sorts samples by treatment so this is valid."""
    nblk = n_samp // SB
    if experts is None:
        experts = ("both",) * nblk
    assert len(experts) == nblk
    nc = bacc.Bacc("TRN2", target_bir_lowering=False, debug=False)

    x_d = nc.dram_tensor("x", [n_samp, D_IN], F32R, kind="ExternalInput")
    t_d = nc.dram_tensor("treat", [n_samp], mybir.dt.int32, kind="ExternalInput")
    w_d = {}
    for name, shape in [
        ("W0", [D_IN, H1]), ("b0", [H1]),
        ("W1", [H1, H2]), ("b1", [H2]),
        ("W2", [H2, D_SH]), ("b2", [D_SH]),
        ("A0", [D_SH, HI]), ("a0", [HI]),
        ("A1", [HI, HI]), ("a1", [HI]),
        ("A2", [D_SH, 1]),
        ("C0", [D_SH, HI]), ("c0", [HI]),
        ("C1", [HI, HI]), ("c1", [HI]),
        ("C2", [D_SH, 1]),
    ]:
        # biases only feed ACT/DVE evacs — keep them plain float32 (DVE
        # tensor_scalar rejects float32r scalar operands)
        dt_ = F32 if name[0].islower() else F32R
        w_d[name] = nc.dram_tensor(name, shape, dt_, kind="ExternalInput")
    id_d = nc.dram_tensor("ident", [P, P], F32R, kind="ExternalInput")

    y_d = nc.dram_tensor("y", [n_samp], F32, kind="ExternalOutput")
    out_d = nc.dram_tensor("out", [n_samp, D_SH], F32R, kind="ExternalOutput")

    K0, K1, K2 = D_IN // P, H1 // P, H2 // P   # 4, 8, 8
    KR = D_SH // P                              # 4 (risk net ktiles)
    M1, M2, M3 = H1 // P, H2 // P, D_SH // P   # 8, 8, 4
    MR = HI // P                                # 4

    AL = mybir.AluOpType

    def evac_relu_bias(dst, ps, bias_col):
        """dst = relu(ps + bias).  DVE keeps ScalarE free for h1/h2 evacs."""
        if dve_evac:
            nc.vector.tensor_scalar(dst, ps, bias_col, 0.0, AL.add, AL.max)
        else:
            nc.scalar.activation(dst, ps, AF.Relu, bias=bias_col)

    def evac_bias(dst, ps, bias_col):
        if dve_evac:
            nc.vector.tensor_scalar_add(dst, ps, bias_col)
        else:
            nc.scalar.activation(dst, ps, AF.Identity, bias=bias_col)

    with tile.TileContext(nc) as tc:
        with (
            tc.tile_pool(name="wp", bufs=1) as wp,
            tc.tile_pool(name="ap", bufs=1) as ap,
            tc.tile_pool(name="xp", bufs=2) as xp,
            tc.tile_pool(name="mmps", bufs=5, space="PSUM") as mmps,
            tc.tile_pool(name="tpps", bufs=3, space="PSUM") as tpps,
        ):
            # ---- resident weights, feature-major natural layout ----
            def load_w(name, k, m):
                t = wp.tile([P, k, m], F32R, tag=name)
                nc.sync.dma_start(
                    t, w_d[name].ap().rearrange("(kt p) m -> p kt m", p=P)
                )
                return t

            def load_b(name, m):
                t = wp.tile([P, m], F32, tag=name)
                nc.sync.dma_start(t, w_d[name].ap().rearrange("(mt p) -> p mt", p=P))
                return t

            # identity + first x block are prefetched ahead of the (large)
            # weight DMAs so the PE can start block-0 transposes immediately.
            ident = wp.tile([P, P], F32R, tag="ident")
            nc.sync.dma_start(ident, id_d.ap())
            xsm0 = None
            if dyn_repeat == 1:
                xsm0 = ap.tile([P, SB // P, D_IN], F32R, tag="xsm")
                nc.sync.dma_start(
                    xsm0, x_d.ap()[0:SB, :].rearrange("(st p) f -> p st f", p=P)
                )

            W0t = load_w("W0", K0, H1)
            b0t = load_b("b0", M1)
            W1t = load_w("W1", K1, H2)
            b1t = load_b("b1", M2)
            W2t = load_w("W2", K2, D_SH)
            b2t = load_b("b2", M3)
            A0t = load_w("A0", KR, HI)
            a0t = load_b("a0", MR)
            A1t = load_w("A1", MR, HI)
            a1t = load_b("a1", MR)
            A2t = load_w("A2", KR, 1)
            C0t = load_w("C0", KR, HI)
            c0t = load_b("c0", MR)
            C1t = load_w("C1", MR, HI)
            c1t = load_b("c1", MR)
            C2t = load_w("C2", KR, 1)

            def emit_block(b, xsm_pre=None):
                mode = experts[b]
                s0 = b * SB
                ST = SB // P  # sample sub-tiles per block

                # ---- load x block (sample-major) and transpose on PE ----
                if xsm_pre is not None:
                    xsm = xsm_pre
                else:
                    xsm = ap.tile([P, ST, D_IN], F32R, tag="xsm")
                    nc.sync.dma_start(
                        xsm,
                        x_d.ap()[s0:s0 + SB, :].rearrange("(st p) f -> p st f", p=P),
                    )
                xT = xp.tile([P, K0, SB], F32R, tag="xT")
                for ft in range(K0):
                    pst = tpps.tile([P, SB], F32R, tag="tp")
                    for st in range(ST):
                        if skip_tp and st > 0:
                            continue  # timing probe: 1 transpose per psum tile
                        nc.tensor.transpose(
                            pst[:, st * P:(st + 1) * P],
                            xsm[:, st, ft * P:(ft + 1) * P], ident
                        )
                    nc.vector.tensor_copy(out=xT[:, ft, :], in_=pst)

                # ---- shared layer 1: h1 = relu(x @ W0 + b0) ----
                h1 = ap.tile([P, M1, SB], F32R, tag="h1")
                for mt in range(M1):
                    ps = mmps.tile([P, SB], F32, tag="ps")
                    for kt in range(K0):
                        nc.tensor.matmul(
                            ps, W0t[:, kt, mt * P:(mt + 1) * P], xT[:, kt, :],
                            start=(kt == 0), stop=(kt == K0 - 1),
                        )
                    nc.scalar.activation(
                        h1[:, mt, :], ps, AF.Relu, bias=b0t[:, mt:mt + 1]
                    )

                # ---- shared layer 2: h2 = relu(h1 @ W1 + b1) ----
                h2 = ap.tile([P, M2, SB], F32R, tag="h2")
                for mt in range(M2):
                    ps = mmps.tile([P, SB], F32, tag="ps")
                    for kt in range(K1):
                        nc.tensor.matmul(
                            ps, W1t[:, kt, mt * P:(mt + 1) * P], h1[:, kt, :],
                            start=(kt == 0), stop=(kt == K1 - 1),
                        )
                    nc.scalar.activation(
                        h2[:, mt, :], ps, AF.Relu, bias=b1t[:, mt:mt + 1]
                    )

                # ---- shared layer 3: out = h2 @ W2 + b2 (linear) ----
                outfm = ap.tile([P, M3, SB], F32R, tag="outfm")
                for mt in range(M3):
                    ps = mmps.tile([P, SB], F32, tag="ps")
                    for kt in range(K2):
                        nc.tensor.matmul(
                            ps, W2t[:, kt, mt * P:(mt + 1) * P], h2[:, kt, :],
                            start=(kt == 0), stop=(kt == K2 - 1),
                        )
                    evac_bias(outfm[:, mt, :], ps, b2t[:, mt:mt + 1])

                # ---- transpose out back to sample-major and store ----
                outsm = ap.tile([P, ST, D_SH], F32R, tag="outsm")
                for st in range(ST):
                    pst = tpps.tile([P, SB], F32R, tag="tp")
                    for ft in range(M3):
                        if skip_tp and ft > 0:
                            continue  # timing probe: 1 transpose per psum tile
                        nc.tensor.transpose(
                            pst[:, ft * P:(ft + 1) * P],
                            outfm[:, ft, st * P:(st + 1) * P], ident
                        )
                    nc.vector.tensor_copy(out=outsm[:, st, :], in_=pst)
                nc.sync.dma_start(
                    out_d.ap()[s0:s0 + SB, :].rearrange("(st p) f -> p st f", p=P),
                    outsm,
                )

                # ---- risk nets (both experts on all samples) ----
                def risk_net(U0, u0, u1t, U1, U2, tag):
                    r1 = ap.tile([P, MR, SB], F32R, tag="r1")
                    for mt in range(MR):
                        ps = mmps.tile([P, SB], F32, tag="ps")
                        for kt in range(KR):
                            nc.tensor.matmul(
                                ps, U0[:, kt, mt * P:(mt + 1) * P], outfm[:, kt, :],
                                start=(kt == 0), stop=(kt == KR - 1),
                            )
                        evac_relu_bias(r1[:, mt, :], ps, u0[:, mt:mt + 1])
                    r2 = ap.tile([P, MR, SB], F32R, tag="r2")
                    for mt in range(MR):
                        ps = mmps.tile([P, SB], F32, tag="ps")
                        for kt in range(MR):
                            nc.tensor.matmul(
                                ps, U1[:, kt, mt * P:(mt + 1) * P], r1[:, kt, :],
                                start=(kt == 0), stop=(kt == MR - 1),
                            )
                        evac_relu_bias(r2[:, mt, :], ps, u1t[:, mt:mt + 1])
                    ysc = mmps.tile([1, SB], F32, tag="ps")
                    for kt in range(MR):
                        nc.tensor.matmul(
                            ysc, U2[:, kt, :], r2[:, kt, :],
                            start=(kt == 0), stop=(kt == MR - 1),
                        )
                    yrow = ap.tile([1, SB], F32, tag=f"yrow{tag}")
                    nc.scalar.activation(yrow, ysc, AF.Copy)
                    return yrow

                if mode == "A":
                    y0row = risk_net(A0t, a0t, a1t, A1t, A2t, "0")
                    nc.sync.dma_start(y_d.ap()[None, s0:s0 + SB], y0row)
                elif mode == "C":
                    y1row = risk_net(C0t, c0t, c1t, C1t, C2t, "1")
                    nc.sync.dma_start(y_d.ap()[None, s0:s0 + SB], y1row)
                else:
                    y0row = risk_net(A0t, a0t, a1t, A1t, A2t, "0")
                    y1row = risk_net(C0t, c0t, c1t, C1t, C2t, "1")
                    # ---- per-sample treatment select ----
                    trow = ap.tile([1, SB], mybir.dt.int32, tag="trow")
                    nc.sync.dma_start(trow, t_d.ap()[None, s0:s0 + SB])
                    tf = ap.tile([1, SB], F32, tag="tf")
                    nc.vector.tensor_copy(out=tf, in_=trow)
                    # y = y0 + t * (y1 - y0)   (t is exactly 0.0 or 1.0)
                    ysel = ap.tile([1, SB], F32, tag="ysel")
                    nc.vector.tensor_sub(out=ysel, in0=y1row, in1=y0row)
                    nc.vector.tensor_mul(out=ysel, in0=ysel, in1=tf)
                    nc.vector.tensor_add(out=ysel, in0=ysel, in1=y0row)
                    nc.sync.dma_start(y_d.ap()[None, s0:s0 + SB], ysel)

            def emit_all(xsm0=None):
                for b in range(nblk * repeat):
                    emit_block(b % nblk, xsm_pre=xsm0 if b == 0 else None)

            if dyn_repeat > 1:
                with tc.For_i(0, dyn_repeat, 1):
                    emit_all()
            else:
                emit_all(xsm0)

    nc.compile()
    return nc


_CACHE = {}


def _get_module(n_samp, experts=None):
    key = (n_samp, experts)
    if key not in _CACHE:
        _CACHE[key] = build_module(n_samp, experts=experts)
    return _CACHE[key]


def _routing_plan(treat, n_core):
    """Per-core stable sort by treatment so most blocks are single-expert.

    Schedule: pa pure-A blocks, 2 mixed blocks, rest pure-C.  Valid iff each
    core's t==0 count lands inside the mixed window — an ~11-sigma certainty
    for balanced random treatment; returns None otherwise (generic fallback).
    """
    nblk = n_core // SB
    if n_core % SB or nblk < 4:
        return None
    pa = nblk // 2 - 1
    lo, hi = pa * SB, (pa + 2) * SB
    perms = []
    for c in range(N_CORES):
        tc_ = treat[c * n_core:(c + 1) * n_core]
        c0 = int((tc_ == 0).sum())
        if not (lo <= c0 <= hi):
            return None
        perms.append(np.argsort(tc_, kind="stable") + c * n_core)
    experts = ("A",) * pa + ("both",) * 2 + ("C",) * (nblk - pa - 2)
    return np.concatenate(perms), experts


def kernel(**inputs):
    x = np.ascontiguousarray(np.asarray(inputs["input"], dtype=np.float32))
    treat = np.ascontiguousarray(np.asarray(inputs["treatment"], dtype=np.int32))
    n = x.shape[0]
    n_core = n // N_CORES

    plan = _routing_plan(treat, n_core)
    if plan is not None:
        perm, experts = plan
        x_k, t_k = x[perm], treat[perm]
    else:
        perm, experts = None, None
        x_k, t_k = x, treat

    nc = _get_module(n_core, experts)

    common = {"ident": np.eye(P, dtype=np.float32)}
    for name in ["W0", "b0", "W1", "b1", "W2", "b2",
                 "A0", "a0", "A1", "a1", "A2", "C0", "c0", "C1", "c1", "C2"]:
        arr = np.ascontiguousarray(np.asarray(inputs[name], dtype=np.float32))
        common[name] = arr

    in_maps = []
    for c in range(N_CORES):
        sl = slice(c * n_core, (c + 1) * n_core)
        m = dict(common)
        m["x"] = np.ascontiguousarray(x_k[sl])
        m["treat"] = np.ascontiguousarray(t_k[sl])
        in_maps.append(m)

    res = run_bass_kernel_spmd(nc, in_maps, core_ids=list(range(N_CORES)))
    y = np.concatenate([r["y"] for r in res.results])
    out = np.concatenate([r["out"] for r in res.results], axis=0)
    if perm is not None:
        y_u = np.empty_like(y)
        out_u = np.empty_like(out)
        y_u[perm] = y
        out_u[perm] = out
        y, out = y_u, out_u
    return y, out


# revision 31
# speedup vs baseline: 1.0556x; 1.0556x over previous
"""CFRNet (moe_routing) Trainium2 Bass kernel.

Shared MLP 512->1024->1024->512 (ReLU,ReLU,linear+bias) followed by two
"risk net" experts 512->512->512->1 with per-sample binary treatment
routing.  Data-parallel across 8 NeuronCores: each core handles 8192 of
the 65536 samples; the (small) weights are replicated.

Design:
  - Activations live feature-major in SBUF: [128 feat_part, ktile, samp].
    Every layer is  out_fm = matmul(lhsT=W_tile, rhs=act_fm)  with weights
    in their natural [in_feat, out_feat] HBM layout and the output again
    feature-major.  The host sends x pre-transposed (feature-major) and
    receives `out` feature-major, so NO on-chip transposes are needed; the
    cheap [65536,512] transposes happen in numpy during shard/unshard.
  - All matmul operands are float32r (reduced-precision fp32): 1 PE pass,
    ~FP22 multiply precision, fp32 PSUM accumulate (float32 would be 4
    passes; bf16 would be no faster than float32r on this PE).
  - Expert routing: the host stable-sorts each core's samples by treatment,
    so most 512-sample blocks are single-treatment and run only one risk
    net.  Schedule (pa pure-A | 2 mixed | pure-C) is data-independent given
    balanced treatment (~11 sigma margin); falls back to both-experts
    everywhere otherwise.
  - PSUM evacuation split across engines: ScalarE handles h1/h2 (Relu+bias
    activation), DVE handles layer-3/risk-net evacs via fused
    tensor_scalar(add,max) so neither engine gates the PE.
"""

import numpy as np

import concourse.mybir as mybir
import concourse.tile as tile
from concourse import bacc
from concourse.bass_utils import run_bass_kernel_spmd

P = 128
N_CORES = 8
N_TOTAL = 65536
N_CORE = N_TOTAL // N_CORES  # 8192
D_IN, H1, H2, D_SH, HI = 512, 1024, 1024, 512, 512
SB = 512  # samples per block
F32R = mybir.dt.float32r
F32 = mybir.dt.float32
AF = mybir.ActivationFunctionType
AL = mybir.AluOpType

WEIGHT_NAMES = ["W0", "b0", "W1", "b1", "W2", "b2",
                "A0", "a0", "A1", "a1", "A2", "C0", "c0", "C1", "c1", "C2"]


def build_module(n_samp=N_CORE, repeat=1, dyn_repeat=1, experts=None):
    """experts: per-block expert schedule, tuple of "A"/"C"/"both" of length
    n_samp//SB (None -> all "both").  Pure blocks run only one risk net —
    valid because the host pre-sorts samples by treatment."""
    nblk = n_samp // SB
    if experts is None:
        experts = ("both",) * nblk
    assert len(experts) == nblk
    nc = bacc.Bacc("TRN2", target_bir_lowering=False, debug=False)

    x_d = nc.dram_tensor("xT", [D_IN, n_samp], F32R, kind="ExternalInput")
    t_d = nc.dram_tensor("treat", [n_samp], mybir.dt.int32, kind="ExternalInput")
    w_d = {}
    for name, shape in [
        ("W0", [D_IN, H1]), ("b0", [H1]),
        ("W1", [H1, H2]), ("b1", [H2]),
        ("W2", [H2, D_SH]), ("b2", [D_SH]),
        ("A0", [D_SH, HI]), ("a0", [HI]),
        ("A1", [HI, HI]), ("a1", [HI]),
        ("A2", [D_SH, 1]),
        ("C0", [D_SH, HI]), ("c0", [HI]),
        ("C1", [HI, HI]), ("c1", [HI]),
        ("C2", [D_SH, 1]),
    ]:
        # biases only feed ACT/DVE evacs — keep them plain float32 (DVE
        # tensor_scalar rejects float32r scalar operands)
        dt_ = F32 if name[0].islower() else F32R
        w_d[name] = nc.dram_tensor(name, shape, dt_, kind="ExternalInput")

    y_d = nc.dram_tensor("y", [n_samp], F32, kind="ExternalOutput")
    out_d = nc.dram_tensor("outT", [D_SH, n_samp], F32R, kind="ExternalOutput")

    K0, K1, K2 = D_IN // P, H1 // P, H2 // P   # 4, 8, 8
    KR = D_SH // P                              # 4 (risk net ktiles)
    M1, M2, M3 = H1 // P, H2 // P, D_SH // P   # 8, 8, 4
    MR = HI // P                                # 4

    x_fm = x_d.ap().rearrange("(kt p) s -> p kt s", p=P)
    out_fm = out_d.ap().rearrange("(kt p) s -> p kt s", p=P)

    with tile.TileContext(nc) as tc:
        with (
            tc.tile_pool(name="wp", bufs=1) as wp,
            tc.tile_pool(name="ap", bufs=1) as ap,
            tc.tile_pool(name="xp", bufs=2) as xp,
            tc.tile_pool(name="mmps", bufs=8, space="PSUM") as mmps,
        ):
            def load_xT(b):
                t = xp.tile([P, K0, SB], F32R, tag="xT")
                nc.sync.dma_start(t, x_fm[:, :, b * SB:(b + 1) * SB])
                return t

            # first x block is prefetched ahead of the (large) weight DMAs
            # so the PE can start block-0 work as early as possible
            xT0 = load_xT(0) if dyn_repeat == 1 else None

            # ---- resident weights, feature-major natural layout ----
            def load_w(name, k, m):
                t = wp.tile([P, k, m], F32R, tag=name)
                nc.sync.dma_start(
                    t, w_d[name].ap().rearrange("(kt p) m -> p kt m", p=P)
                )
                return t

            def load_b(name, m):
                t = wp.tile([P, m], F32, tag=name)
                nc.sync.dma_start(t, w_d[name].ap().rearrange("(mt p) -> p mt", p=P))
                return t

            W0t = load_w("W0", K0, H1)
            b0t = load_b("b0", M1)
            W1t = load_w("W1", K1, H2)
            b1t = load_b("b1", M2)
            W2t = load_w("W2", K2, D_SH)
            b2t = load_b("b2", M3)
            A0t = load_w("A0", KR, HI)
            a0t = load_b("a0", MR)
            A1t = load_w("A1", MR, HI)
            a1t = load_b("a1", MR)
            A2t = load_w("A2", KR, 1)
            C0t = load_w("C0", KR, HI)
            c0t = load_b("c0", MR)
            C1t = load_w("C1", MR, HI)
            c1t = load_b("c1", MR)
            C2t = load_w("C2", KR, 1)

            def emit_block(b, xT_pre=None):
                mode = experts[b]
                s0 = b * SB

                xT = xT_pre if xT_pre is not None else load_xT(b)

                # ---- shared layer 1: h1 = relu(x @ W0 + b0) ----
                h1 = ap.tile([P, M1, SB], F32R, tag="h1")
                for mt in range(M1):
                    ps = mmps.tile([P, SB], F32, tag="ps")
                    for kt in range(K0):
                        nc.tensor.matmul(
                            ps, W0t[:, kt, mt * P:(mt + 1) * P], xT[:, kt, :],
                            start=(kt == 0), stop=(kt == K0 - 1),
                        )
                    nc.scalar.activation(
                        h1[:, mt, :], ps, AF.Relu, bias=b0t[:, mt:mt + 1]
                    )

                # ---- shared layer 2: h2 = relu(h1 @ W1 + b1) ----
                h2 = ap.tile([P, M2, SB], F32R, tag="h2")
                for mt in range(M2):
                    ps = mmps.tile([P, SB], F32, tag="ps")
                    for kt in range(K1):
                        nc.tensor.matmul(
                            ps, W1t[:, kt, mt * P:(mt + 1) * P], h1[:, kt, :],
                            start=(kt == 0), stop=(kt == K1 - 1),
                        )
                    nc.scalar.activation(
                        h2[:, mt, :], ps, AF.Relu, bias=b1t[:, mt:mt + 1]
                    )

                # ---- shared layer 3: out = h2 @ W2 + b2 (linear), stored
                # feature-major straight to HBM ----
                outfm = ap.tile([P, M3, SB], F32R, tag="outfm")
                for mt in range(M3):
                    ps = mmps.tile([P, SB], F32, tag="ps")
                    for kt in range(K2):
                        nc.tensor.matmul(
                            ps, W2t[:, kt, mt * P:(mt + 1) * P], h2[:, kt, :],
                            start=(kt == 0), stop=(kt == K2 - 1),
                        )
                    nc.vector.tensor_scalar_add(
                        outfm[:, mt, :], ps, b2t[:, mt:mt + 1]
                    )
                nc.sync.dma_start(out_fm[:, :, s0:s0 + SB], outfm)

                # ---- risk nets ----
                def risk_net(U0, u0, U1, u1, U2, tag):
                    r1 = ap.tile([P, MR, SB], F32R, tag="r1")
                    for mt in range(MR):
                        ps = mmps.tile([P, SB], F32, tag="ps")
                        for kt in range(KR):
                            nc.tensor.matmul(
                                ps, U0[:, kt, mt * P:(mt + 1) * P], outfm[:, kt, :],
                                start=(kt == 0), stop=(kt == KR - 1),
                            )
                        nc.vector.tensor_scalar(
                            r1[:, mt, :], ps, u0[:, mt:mt + 1], 0.0, AL.add, AL.max
                        )
                    r2 = ap.tile([P, MR, SB], F32R, tag="r2")
                    for mt in range(MR):
                        ps = mmps.tile([P, SB], F32, tag="ps")
                        for kt in range(MR):
                            nc.tensor.matmul(
                                ps, U1[:, kt, mt * P:(mt + 1) * P], r1[:, kt, :],
                                start=(kt == 0), stop=(kt == MR - 1),
                            )
                        nc.vector.tensor_scalar(
                            r2[:, mt, :], ps, u1[:, mt:mt + 1], 0.0, AL.add, AL.max
                        )
                    ysc = mmps.tile([1, SB], F32, tag="ps")
                    for kt in range(MR):
                        nc.tensor.matmul(
                            ysc, U2[:, kt, :], r2[:, kt, :],
                            start=(kt == 0), stop=(kt == MR - 1),
                        )
                    yrow = ap.tile([1, SB], F32, tag=f"yrow{tag}")
                    nc.scalar.activation(yrow, ysc, AF.Copy)
                    return yrow

                if mode == "A":
                    y0row = risk_net(A0t, a0t, A1t, a1t, A2t, "0")
                    nc.sync.dma_start(y_d.ap()[None, s0:s0 + SB], y0row)
                elif mode == "C":
                    y1row = risk_net(C0t, c0t, C1t, c1t, C2t, "1")
                    nc.sync.dma_start(y_d.ap()[None, s0:s0 + SB], y1row)
                else:
                    y0row = risk_net(A0t, a0t, A1t, a1t, A2t, "0")
                    y1row = risk_net(C0t, c0t, C1t, c1t, C2t, "1")
                    # per-sample treatment select: y = y0 + t*(y1-y0)
                    trow = ap.tile([1, SB], mybir.dt.int32, tag="trow")
                    nc.sync.dma_start(trow, t_d.ap()[None, s0:s0 + SB])
                    tf = ap.tile([1, SB], F32, tag="tf")
                    nc.vector.tensor_copy(out=tf, in_=trow)
                    ysel = ap.tile([1, SB], F32, tag="ysel")
                    nc.vector.tensor_sub(out=ysel, in0=y1row, in1=y0row)
                    nc.vector.tensor_mul(out=ysel, in0=ysel, in1=tf)
                    nc.vector.tensor_add(out=ysel, in0=ysel, in1=y0row)
                    nc.sync.dma_start(y_d.ap()[None, s0:s0 + SB], ysel)

            def emit_all(xT0=None):
                for b in range(nblk * repeat):
                    emit_block(b % nblk, xT_pre=xT0 if b == 0 else None)

            if dyn_repeat > 1:
                with tc.For_i(0, dyn_repeat, 1):
                    emit_all()
            else:
                emit_all(xT0)

    nc.compile()
    return nc


_CACHE = {}


def _get_module(n_samp, experts=None):
    key = (n_samp, experts)
    if key not in _CACHE:
        _CACHE[key] = build_module(n_samp, experts=experts)
    return _CACHE[key]


def _routing_plan(treat, n_core):
    """Per-core stable sort by treatment so most blocks are single-expert.

    Schedule: pa pure-A blocks, 2 mixed blocks, rest pure-C.  Valid iff each
    core's t==0 count lands inside the mixed window — an ~11-sigma certainty
    for balanced random treatment; returns None otherwise (generic fallback).
    """
    nblk = n_core // SB
    if n_core % SB or nblk < 4:
        return None
    pa = nblk // 2 - 1
    lo, hi = pa * SB, (pa + 2) * SB
    perms = []
    for c in range(N_CORES):
        tc_ = treat[c * n_core:(c + 1) * n_core]
        c0 = int((tc_ == 0).sum())
        if not (lo <= c0 <= hi):
            return None
        perms.append(np.argsort(tc_, kind="stable") + c * n_core)
    experts = ("A",) * pa + ("both",) * 2 + ("C",) * (nblk - pa - 2)
    return np.concatenate(perms), experts


def kernel(**inputs):
    x = np.ascontiguousarray(np.asarray(inputs["input"], dtype=np.float32))
    treat = np.ascontiguousarray(np.asarray(inputs["treatment"], dtype=np.int32))
    n = x.shape[0]
    n_core = n // N_CORES

    plan = _routing_plan(treat, n_core)
    if plan is not None:
        perm, experts = plan
        x_k, t_k = x[perm], treat[perm]
    else:
        perm, experts = None, None
        x_k, t_k = x, treat

    nc = _get_module(n_core, experts)

    common = {}
    for name in WEIGHT_NAMES:
        common[name] = np.ascontiguousarray(np.asarray(inputs[name], np.float32))

    in_maps = []
    for c in range(N_CORES):
        sl = slice(c * n_core, (c + 1) * n_core)
        m = dict(common)
        m["xT"] = np.ascontiguousarray(x_k[sl].T)
        m["treat"] = np.ascontiguousarray(t_k[sl])
        in_maps.append(m)

    res = run_bass_kernel_spmd(nc, in_maps, core_ids=list(range(N_CORES)))
    y = np.concatenate([r["y"] for r in res.results])
    out = np.concatenate([np.ascontiguousarray(r["outT"].T)
                          for r in res.results], axis=0)
    if perm is not None:
        y_u = np.empty_like(y)
        out_u = np.empty_like(out)
        y_u[perm] = y
        out_u[perm] = out
        y, out = y_u, out_u
    return y, out


# revision 35
# speedup vs baseline: 1.0710x; 1.0146x over previous
"""CFRNet (moe_routing) Trainium2 Bass kernel.

Shared MLP 512->1024->1024->512 (ReLU,ReLU,linear+bias) followed by two
"risk net" experts 512->512->512->1 with per-sample binary treatment
routing.  Data-parallel across 8 NeuronCores: each core handles 8192 of
the 65536 samples; the (small) weights are replicated.

Design:
  - Activations live feature-major in SBUF: [128 feat_part, ktile, samp].
    Every layer is  out_fm = matmul(lhsT=W_tile, rhs=act_fm)  with weights
    in their natural [in_feat, out_feat] HBM layout and the output again
    feature-major.  The host sends x pre-transposed (feature-major) and
    receives `out` feature-major, so NO on-chip transposes are needed; the
    cheap [65536,512] transposes happen in numpy during shard/unshard.
  - All matmul operands are float32r (reduced-precision fp32): 1 PE pass,
    ~FP22 multiply precision, fp32 PSUM accumulate (float32 would be 4
    passes; bf16 would be no faster than float32r on this PE).
  - Expert routing: the host stable-sorts each core's samples by treatment,
    so most 512-sample blocks are single-treatment and run only one risk
    net.  Schedule (pa pure-A | 2 mixed | pure-C) is data-independent given
    balanced treatment (~11 sigma margin); falls back to both-experts
    everywhere otherwise.
  - PSUM evacuation split across engines: ScalarE handles h1/h2 (Relu+bias
    activation), DVE handles layer-3/risk-net evacs via fused
    tensor_scalar(add,max) so neither engine gates the PE.
"""

import numpy as np

import concourse.mybir as mybir
import concourse.tile as tile
from concourse import bacc
from concourse.bass_utils import run_bass_kernel_spmd

P = 128
N_CORES = 8
N_TOTAL = 65536
N_CORE = N_TOTAL // N_CORES  # 8192
D_IN, H1, H2, D_SH, HI = 512, 1024, 1024, 512, 512
SB = 512  # samples per block
F32R = mybir.dt.float32r
F32 = mybir.dt.float32
AF = mybir.ActivationFunctionType
AL = mybir.AluOpType

WEIGHT_NAMES = ["W0", "b0", "W1", "b1", "W2", "b2",
                "A0", "a0", "A1", "a1", "A2", "C0", "c0", "C1", "c1", "C2"]


def build_module(n_samp=N_CORE, repeat=1, dyn_repeat=1, experts=None,
                 h_evac="act"):
    """experts: per-block expert schedule, tuple of "A"/"C"/"both" of length
    n_samp//SB (None -> all "both").  Pure blocks run only one risk net —
    valid because the host pre-sorts samples by treatment."""
    nblk = n_samp // SB
    if experts is None:
        experts = ("both",) * nblk
    assert len(experts) == nblk
    nc = bacc.Bacc("TRN2", target_bir_lowering=False, debug=False)

    x_d = nc.dram_tensor("xT", [D_IN, n_samp], F32R, kind="ExternalInput")
    t_d = nc.dram_tensor("treat", [n_samp], mybir.dt.int32, kind="ExternalInput")
    w_d = {}
    for name, shape in [
        ("W0", [D_IN, H1]), ("b0", [H1]),
        ("W1", [H1, H2]), ("b1", [H2]),
        ("W2", [H2, D_SH]), ("b2", [D_SH]),
        ("A0", [D_SH, HI]), ("a0", [HI]),
        ("A1", [HI, HI]), ("a1", [HI]),
        ("A2", [D_SH, 1]),
        ("C0", [D_SH, HI]), ("c0", [HI]),
        ("C1", [HI, HI]), ("c1", [HI]),
        ("C2", [D_SH, 1]),
    ]:
        # biases only feed ACT/DVE evacs — keep them plain float32 (DVE
        # tensor_scalar rejects float32r scalar operands)
        dt_ = F32 if name[0].islower() else F32R
        w_d[name] = nc.dram_tensor(name, shape, dt_, kind="ExternalInput")

    y_d = nc.dram_tensor("y", [n_samp], F32, kind="ExternalOutput")
    out_d = nc.dram_tensor("outT", [D_SH, n_samp], F32R, kind="ExternalOutput")

    K0, K1, K2 = D_IN // P, H1 // P, H2 // P   # 4, 8, 8
    KR = D_SH // P                              # 4 (risk net ktiles)
    M1, M2, M3 = H1 // P, H2 // P, D_SH // P   # 8, 8, 4
    MR = HI // P                                # 4

    x_fm = x_d.ap().rearrange("(kt p) s -> p kt s", p=P)
    out_fm = out_d.ap().rearrange("(kt p) s -> p kt s", p=P)

    with tile.TileContext(nc) as tc:
        with (
            tc.tile_pool(name="wp", bufs=1) as wp,
            tc.tile_pool(name="ap", bufs=1) as ap,
            tc.tile_pool(name="xp", bufs=2) as xp,
            tc.tile_pool(name="mmps", bufs=8, space="PSUM") as mmps,
        ):
            def load_xT(b):
                t = xp.tile([P, K0, SB], F32R, tag="xT")
                nc.sync.dma_start(t, x_fm[:, :, b * SB:(b + 1) * SB])
                return t

            # first x block is prefetched ahead of the (large) weight DMAs
            # so the PE can start block-0 work as early as possible
            xT0 = load_xT(0) if dyn_repeat == 1 else None

            # ---- resident weights, feature-major natural layout ----
            def load_w(name, k, m):
                t = wp.tile([P, k, m], F32R, tag=name)
                nc.sync.dma_start(
                    t, w_d[name].ap().rearrange("(kt p) m -> p kt m", p=P)
                )
                return t

            def load_b(name, m):
                t = wp.tile([P, m], F32, tag=name)
                nc.sync.dma_start(t, w_d[name].ap().rearrange("(mt p) -> p mt", p=P))
                return t

            W0t = load_w("W0", K0, H1)
            b0t = load_b("b0", M1)
            W1t = load_w("W1", K1, H2)
            b1t = load_b("b1", M2)
            W2t = load_w("W2", K2, D_SH)
            b2t = load_b("b2", M3)
            A0t = load_w("A0", KR, HI)
            a0t = load_b("a0", MR)
            A1t = load_w("A1", MR, HI)
            a1t = load_b("a1", MR)
            A2t = load_w("A2", KR, 1)
            C0t = load_w("C0", KR, HI)
            c0t = load_b("c0", MR)
            C1t = load_w("C1", MR, HI)
            c1t = load_b("c1", MR)
            C2t = load_w("C2", KR, 1)

            def h_relu_evac(dst, ps, bias_col, mt):
                use_dve = h_evac == "dve" or (h_evac == "alt" and mt % 2)
                if use_dve:
                    nc.vector.tensor_scalar(dst, ps, bias_col, 0.0, AL.add, AL.max)
                else:
                    nc.scalar.activation(dst, ps, AF.Relu, bias=bias_col)

            def emit_block(b, xT_pre=None):
                mode = experts[b]
                s0 = b * SB

                xT = xT_pre if xT_pre is not None else load_xT(b)

                # ---- shared layer 1: h1 = relu(x @ W0 + b0) ----
                h1 = ap.tile([P, M1, SB], F32R, tag="h1")
                for mt in range(M1):
                    ps = mmps.tile([P, SB], F32, tag="ps")
                    for kt in range(K0):
                        nc.tensor.matmul(
                            ps, W0t[:, kt, mt * P:(mt + 1) * P], xT[:, kt, :],
                            start=(kt == 0), stop=(kt == K0 - 1),
                        )
                    h_relu_evac(h1[:, mt, :], ps, b0t[:, mt:mt + 1], mt)

                # ---- shared layer 2: h2 = relu(h1 @ W1 + b1) ----
                h2 = ap.tile([P, M2, SB], F32R, tag="h2")
                for mt in range(M2):
                    ps = mmps.tile([P, SB], F32, tag="ps")
                    for kt in range(K1):
                        nc.tensor.matmul(
                            ps, W1t[:, kt, mt * P:(mt + 1) * P], h1[:, kt, :],
                            start=(kt == 0), stop=(kt == K1 - 1),
                        )
                    h_relu_evac(h2[:, mt, :], ps, b1t[:, mt:mt + 1], mt)

                # ---- shared layer 3: out = h2 @ W2 + b2 (linear), stored
                # feature-major straight to HBM ----
                outfm = ap.tile([P, M3, SB], F32R, tag="outfm")
                for mt in range(M3):
                    ps = mmps.tile([P, SB], F32, tag="ps")
                    for kt in range(K2):
                        nc.tensor.matmul(
                            ps, W2t[:, kt, mt * P:(mt + 1) * P], h2[:, kt, :],
                            start=(kt == 0), stop=(kt == K2 - 1),
                        )
                    nc.vector.tensor_scalar_add(
                        outfm[:, mt, :], ps, b2t[:, mt:mt + 1]
                    )
                nc.sync.dma_start(out_fm[:, :, s0:s0 + SB], outfm)

                # ---- risk nets ----
                def risk_net(U0, u0, U1, u1, U2, tag):
                    r1 = ap.tile([P, MR, SB], F32R, tag="r1")
                    for mt in range(MR):
                        ps = mmps.tile([P, SB], F32, tag="ps")
                        for kt in range(KR):
                            nc.tensor.matmul(
                                ps, U0[:, kt, mt * P:(mt + 1) * P], outfm[:, kt, :],
                                start=(kt == 0), stop=(kt == KR - 1),
                            )
                        nc.vector.tensor_scalar(
                            r1[:, mt, :], ps, u0[:, mt:mt + 1], 0.0, AL.add, AL.max
                        )
                    r2 = ap.tile([P, MR, SB], F32R, tag="r2")
                    for mt in range(MR):
                        ps = mmps.tile([P, SB], F32, tag="ps")
                        for kt in range(MR):
                            nc.tensor.matmul(
                                ps, U1[:, kt, mt * P:(mt + 1) * P], r1[:, kt, :],
                                start=(kt == 0), stop=(kt == MR - 1),
                            )
                        nc.vector.tensor_scalar(
                            r2[:, mt, :], ps, u1[:, mt:mt + 1], 0.0, AL.add, AL.max
                        )
                    ysc = mmps.tile([1, SB], F32, tag="ps")
                    for kt in range(MR):
                        nc.tensor.matmul(
                            ysc, U2[:, kt, :], r2[:, kt, :],
                            start=(kt == 0), stop=(kt == MR - 1),
                        )
                    yrow = ap.tile([1, SB], F32, tag=f"yrow{tag}")
                    nc.scalar.activation(yrow, ysc, AF.Copy)
                    return yrow

                if mode == "A":
                    y0row = risk_net(A0t, a0t, A1t, a1t, A2t, "0")
                    nc.sync.dma_start(y_d.ap()[None, s0:s0 + SB], y0row)
                elif mode == "C":
                    y1row = risk_net(C0t, c0t, C1t, c1t, C2t, "1")
                    nc.sync.dma_start(y_d.ap()[None, s0:s0 + SB], y1row)
                else:
                    y0row = risk_net(A0t, a0t, A1t, a1t, A2t, "0")
                    y1row = risk_net(C0t, c0t, C1t, c1t, C2t, "1")
                    # per-sample treatment select: y = y0 + t*(y1-y0)
                    trow = ap.tile([1, SB], mybir.dt.int32, tag="trow")
                    nc.sync.dma_start(trow, t_d.ap()[None, s0:s0 + SB])
                    tf = ap.tile([1, SB], F32, tag="tf")
                    nc.vector.tensor_copy(out=tf, in_=trow)
                    ysel = ap.tile([1, SB], F32, tag="ysel")
                    nc.vector.tensor_sub(out=ysel, in0=y1row, in1=y0row)
                    nc.vector.tensor_mul(out=ysel, in0=ysel, in1=tf)
                    nc.vector.tensor_add(out=ysel, in0=ysel, in1=y0row)
                    nc.sync.dma_start(y_d.ap()[None, s0:s0 + SB], ysel)

            def emit_all(xT0=None):
                for b in range(nblk * repeat):
                    emit_block(b % nblk, xT_pre=xT0 if b == 0 else None)

            if dyn_repeat > 1:
                with tc.For_i(0, dyn_repeat, 1):
                    emit_all()
            else:
                emit_all(xT0)

    nc.compile()
    return nc


_CACHE = {}


def _get_module(n_samp, experts=None):
    key = (n_samp, experts)
    if key not in _CACHE:
        _CACHE[key] = build_module(n_samp, experts=experts)
    return _CACHE[key]


def _routing_plan(treat, n_core):
    """Per-core stable sort by treatment so most blocks are single-expert.

    Schedule: pa pure-A blocks, 2 mixed blocks, rest pure-C.  Valid iff each
    core's t==0 count lands inside the mixed window — an ~11-sigma certainty
    for balanced random treatment; returns None otherwise (generic fallback).
    """
    nblk = n_core // SB
    if n_core % SB or nblk < 4:
        return None
    pa = nblk // 2 - 1
    lo, hi = pa * SB, (pa + 2) * SB
    perms = []
    for c in range(N_CORES):
        tc_ = treat[c * n_core:(c + 1) * n_core]
        c0 = int((tc_ == 0).sum())
        if not (lo <= c0 <= hi):
            return None
        perms.append(np.argsort(tc_, kind="stable") + c * n_core)
    experts = ("A",) * pa + ("both",) * 2 + ("C",) * (nblk - pa - 2)
    return np.concatenate(perms), experts


def kernel(**inputs):
    x = np.ascontiguousarray(np.asarray(inputs["input"], dtype=np.float32))
    treat = np.ascontiguousarray(np.asarray(inputs["treatment"], dtype=np.int32))
    n = x.shape[0]
    n_core = n // N_CORES

    plan = _routing_plan(treat, n_core)
    if plan is not None:
        perm, experts = plan
        x_k, t_k = x[perm], treat[perm]
    else:
        perm, experts = None, None
        x_k, t_k = x, treat

    nc = _get_module(n_core, experts)

    common = {}
    for name in WEIGHT_NAMES:
        common[name] = np.ascontiguousarray(np.asarray(inputs[name], np.float32))

    in_maps = []
    for c in range(N_CORES):
        sl = slice(c * n_core, (c + 1) * n_core)
        m = dict(common)
        m["xT"] = np.ascontiguousarray(x_k[sl].T)
        m["treat"] = np.ascontiguousarray(t_k[sl])
        in_maps.append(m)

    res = run_bass_kernel_spmd(nc, in_maps, core_ids=list(range(N_CORES)))
    y = np.concatenate([r["y"] for r in res.results])
    out = np.concatenate([np.ascontiguousarray(r["outT"].T)
                          for r in res.results], axis=0)
    if perm is not None:
        y_u = np.empty_like(y)
        out_u = np.empty_like(out)
        y_u[perm] = y
        out_u[perm] = out
        y, out = y_u, out_u
    return y, out


# revision 43
# speedup vs baseline: 1.0884x; 1.0163x over previous
"""CFRNet (moe_routing) Trainium2 Bass kernel.

Shared MLP 512->1024->1024->512 (ReLU,ReLU,linear+bias) followed by two
"risk net" experts 512->512->512->1 with per-sample binary treatment
routing.  Data-parallel across 8 NeuronCores: each core handles 8192 of
the 65536 samples; the (small) weights are replicated.

Design:
  - Activations live feature-major in SBUF: [128 feat_part, ktile, samp].
    Every layer is  out_fm = matmul(lhsT=W_tile, rhs=act_fm)  with weights
    in their natural [in_feat, out_feat] HBM layout and the output again
    feature-major.  The host sends x pre-transposed (feature-major) and
    receives `out` feature-major, so NO on-chip transposes are needed; the
    cheap [65536,512] transposes happen in numpy during shard/unshard.
  - All matmul operands are float32r (reduced-precision fp32): 1 PE pass,
    ~FP22 multiply precision, fp32 PSUM accumulate (float32 would be 4
    passes; bf16 would be no faster than float32r on this PE).
  - Expert routing: the host stable-sorts each core's samples by treatment,
    so most 512-sample blocks are single-treatment and run only one risk
    net.  Schedule (pa pure-A | 2 mixed | pure-C) is data-independent given
    balanced treatment (~11 sigma margin); falls back to both-experts
    everywhere otherwise.
  - PSUM evacuation split across engines: ScalarE handles h1/h2 (Relu+bias
    activation), DVE handles layer-3/risk-net evacs via fused
    tensor_scalar(add,max) so neither engine gates the PE.
"""

import numpy as np

import concourse.mybir as mybir
import concourse.tile as tile
from concourse import bacc
from concourse.bass_utils import run_bass_kernel_spmd

P = 128
N_CORES = 8
N_TOTAL = 65536
N_CORE = N_TOTAL // N_CORES  # 8192
D_IN, H1, H2, D_SH, HI = 512, 1024, 1024, 512, 512
SB = 512  # samples per block
F32R = mybir.dt.float32r
F32 = mybir.dt.float32
AF = mybir.ActivationFunctionType
AL = mybir.AluOpType

WEIGHT_NAMES = ["W0", "b0", "W1", "b1", "W2", "b2",
                "A0", "a0", "A1", "a1", "A2", "C0", "c0", "C1", "c1", "C2"]


def build_module(n_samp=N_CORE, repeat=1, dyn_repeat=1, experts=None,
                 h_evac="act", y_copy="dve"):
    """experts: per-block expert schedule, tuple of "A"/"C"/"both" of length
    n_samp//SB (None -> all "both").  Pure blocks run only one risk net —
    valid because the host pre-sorts samples by treatment."""
    nblk = n_samp // SB
    if experts is None:
        experts = ("both",) * nblk
    assert len(experts) == nblk
    nc = bacc.Bacc("TRN2", target_bir_lowering=False, debug=False)

    x_d = nc.dram_tensor("xT", [D_IN, n_samp], F32R, kind="ExternalInput")
    t_d = nc.dram_tensor("treat", [n_samp], mybir.dt.int32, kind="ExternalInput")
    w_d = {}
    for name, shape in [
        ("W0", [D_IN, H1]), ("b0", [H1]),
        ("W1", [H1, H2]), ("b1", [H2]),
        ("W2", [H2, D_SH]), ("b2", [D_SH]),
        ("A0", [D_SH, HI]), ("a0", [HI]),
        ("A1", [HI, HI]), ("a1", [HI]),
        ("A2", [D_SH, 1]),
        ("C0", [D_SH, HI]), ("c0", [HI]),
        ("C1", [HI, HI]), ("c1", [HI]),
        ("C2", [D_SH, 1]),
    ]:
        # biases only feed ACT/DVE evacs — keep them plain float32 (DVE
        # tensor_scalar rejects float32r scalar operands)
        dt_ = F32 if name[0].islower() else F32R
        w_d[name] = nc.dram_tensor(name, shape, dt_, kind="ExternalInput")

    y_d = nc.dram_tensor("y", [n_samp], F32, kind="ExternalOutput")
    out_d = nc.dram_tensor("outT", [D_SH, n_samp], F32R, kind="ExternalOutput")

    K0, K1, K2 = D_IN // P, H1 // P, H2 // P   # 4, 8, 8
    KR = D_SH // P                              # 4 (risk net ktiles)
    M1, M2, M3 = H1 // P, H2 // P, D_SH // P   # 8, 8, 4
    MR = HI // P                                # 4

    x_fm = x_d.ap().rearrange("(kt p) s -> p kt s", p=P)
    out_fm = out_d.ap().rearrange("(kt p) s -> p kt s", p=P)

    with tile.TileContext(nc) as tc:
        with (
            tc.tile_pool(name="wp", bufs=1) as wp,
            tc.tile_pool(name="ap", bufs=1) as ap,
            tc.tile_pool(name="xp", bufs=2) as xp,
            tc.tile_pool(name="mmps", bufs=8, space="PSUM") as mmps,
        ):
            def load_xT(b):
                t = xp.tile([P, K0, SB], F32R, tag="xT")
                nc.sync.dma_start(t, x_fm[:, :, b * SB:(b + 1) * SB])
                return t

            # first x block is prefetched ahead of the (large) weight DMAs
            # so the PE can start block-0 work as early as possible
            xT0 = load_xT(0) if dyn_repeat == 1 else None

            # ---- resident weights, feature-major natural layout ----
            def load_w(name, k, m, split=1):
                # split>1 loads the weight in mtile-order slices so block-0's
                # first psum groups can start before the whole matrix lands
                t = wp.tile([P, k, m], F32R, tag=name)
                src = w_d[name].ap().rearrange("(kt p) m -> p kt m", p=P)
                step = m // split
                for i in range(split):
                    nc.sync.dma_start(
                        t[:, :, i * step:(i + 1) * step],
                        src[:, :, i * step:(i + 1) * step],
                    )
                return t

            def load_b(name, m):
                t = wp.tile([P, m], F32, tag=name)
                nc.sync.dma_start(t, w_d[name].ap().rearrange("(mt p) -> p mt", p=P))
                return t

            b0t = load_b("b0", M1)
            W0t = load_w("W0", K0, H1, split=8)
            b1t = load_b("b1", M2)
            W1t = load_w("W1", K1, H2, split=8)
            W2t = load_w("W2", K2, D_SH)
            b2t = load_b("b2", M3)
            A0t = load_w("A0", KR, HI)
            a0t = load_b("a0", MR)
            A1t = load_w("A1", MR, HI)
            a1t = load_b("a1", MR)
            A2t = load_w("A2", KR, 1)
            C0t = load_w("C0", KR, HI)
            c0t = load_b("c0", MR)
            C1t = load_w("C1", MR, HI)
            c1t = load_b("c1", MR)
            C2t = load_w("C2", KR, 1)

            def h_relu_evac(dst, ps, bias_col, mt):
                use_dve = h_evac == "dve" or (h_evac == "alt" and mt % 2)
                if use_dve:
                    nc.vector.tensor_scalar(dst, ps, bias_col, 0.0, AL.add, AL.max)
                else:
                    nc.scalar.activation(dst, ps, AF.Relu, bias=bias_col)

            def emit_block(b, xT_pre=None):
                mode = experts[b]
                s0 = b * SB

                xT = xT_pre if xT_pre is not None else load_xT(b)

                # ---- shared layer 1: h1 = relu(x @ W0 + b0) ----
                h1 = ap.tile([P, M1, SB], F32R, tag="h1")
                for mt in range(M1):
                    ps = mmps.tile([P, SB], F32, tag="ps")
                    for kt in range(K0):
                        nc.tensor.matmul(
                            ps, W0t[:, kt, mt * P:(mt + 1) * P], xT[:, kt, :],
                            start=(kt == 0), stop=(kt == K0 - 1),
                        )
                    h_relu_evac(h1[:, mt, :], ps, b0t[:, mt:mt + 1], mt)

                # ---- shared layer 2: h2 = relu(h1 @ W1 + b1) ----
                h2 = ap.tile([P, M2, SB], F32R, tag="h2")
                for mt in range(M2):
                    ps = mmps.tile([P, SB], F32, tag="ps")
                    for kt in range(K1):
                        nc.tensor.matmul(
                            ps, W1t[:, kt, mt * P:(mt + 1) * P], h1[:, kt, :],
                            start=(kt == 0), stop=(kt == K1 - 1),
                        )
                    h_relu_evac(h2[:, mt, :], ps, b1t[:, mt:mt + 1], mt)

                # ---- shared layer 3: out = h2 @ W2 + b2 (linear), stored
                # feature-major straight to HBM ----
                outfm = ap.tile([P, M3, SB], F32R, tag="outfm")
                for mt in range(M3):
                    ps = mmps.tile([P, SB], F32, tag="ps")
                    for kt in range(K2):
                        nc.tensor.matmul(
                            ps, W2t[:, kt, mt * P:(mt + 1) * P], h2[:, kt, :],
                            start=(kt == 0), stop=(kt == K2 - 1),
                        )
                    nc.vector.tensor_scalar_add(
                        outfm[:, mt, :], ps, b2t[:, mt:mt + 1]
                    )
                nc.sync.dma_start(out_fm[:, :, s0:s0 + SB], outfm)

                # ---- risk nets ----
                def risk_net(U0, u0, U1, u1, U2, tag):
                    r1 = ap.tile([P, MR, SB], F32R, tag="r1")
                    for mt in range(MR):
                        ps = mmps.tile([P, SB], F32, tag="ps")
                        for kt in range(KR):
                            nc.tensor.matmul(
                                ps, U0[:, kt, mt * P:(mt + 1) * P], outfm[:, kt, :],
                                start=(kt == 0), stop=(kt == KR - 1),
                            )
                        nc.vector.tensor_scalar(
                            r1[:, mt, :], ps, u0[:, mt:mt + 1], 0.0, AL.add, AL.max
                        )
                    r2 = ap.tile([P, MR, SB], F32R, tag="r2")
                    for mt in range(MR):
                        ps = mmps.tile([P, SB], F32, tag="ps")
                        for kt in range(MR):
                            nc.tensor.matmul(
                                ps, U1[:, kt, mt * P:(mt + 1) * P], r1[:, kt, :],
                                start=(kt == 0), stop=(kt == MR - 1),
                            )
                        nc.vector.tensor_scalar(
                            r2[:, mt, :], ps, u1[:, mt:mt + 1], 0.0, AL.add, AL.max
                        )
                    ysc = mmps.tile([1, SB], F32, tag="ps")
                    for kt in range(MR):
                        nc.tensor.matmul(
                            ysc, U2[:, kt, :], r2[:, kt, :],
                            start=(kt == 0), stop=(kt == MR - 1),
                        )
                    yrow = ap.tile([1, SB], F32, tag=f"yrow{tag}")
                    if y_copy == "dve":
                        # keep ScalarE mono-function (Relu only): a Copy here
                        # would force an ACT table reload every block
                        nc.vector.tensor_copy(out=yrow, in_=ysc)
                    else:
                        nc.scalar.activation(yrow, ysc, AF.Copy)
                    return yrow

                if mode == "A":
                    y0row = risk_net(A0t, a0t, A1t, a1t, A2t, "0")
                    nc.sync.dma_start(y_d.ap()[None, s0:s0 + SB], y0row)
                elif mode == "C":
                    y1row = risk_net(C0t, c0t, C1t, c1t, C2t, "1")
                    nc.sync.dma_start(y_d.ap()[None, s0:s0 + SB], y1row)
                else:
                    y0row = risk_net(A0t, a0t, A1t, a1t, A2t, "0")
                    y1row = risk_net(C0t, c0t, C1t, c1t, C2t, "1")
                    # per-sample treatment select: y = y0 + t*(y1-y0)
                    trow = ap.tile([1, SB], mybir.dt.int32, tag="trow")
                    nc.sync.dma_start(trow, t_d.ap()[None, s0:s0 + SB])
                    tf = ap.tile([1, SB], F32, tag="tf")
                    nc.vector.tensor_copy(out=tf, in_=trow)
                    ysel = ap.tile([1, SB], F32, tag="ysel")
                    nc.vector.tensor_sub(out=ysel, in0=y1row, in1=y0row)
                    nc.vector.tensor_mul(out=ysel, in0=ysel, in1=tf)
                    nc.vector.tensor_add(out=ysel, in0=ysel, in1=y0row)
                    nc.sync.dma_start(y_d.ap()[None, s0:s0 + SB], ysel)

            def emit_all(xT0=None):
                for b in range(nblk * repeat):
                    emit_block(b % nblk, xT_pre=xT0 if b == 0 else None)

            if dyn_repeat > 1:
                with tc.For_i(0, dyn_repeat, 1):
                    emit_all()
            else:
                emit_all(xT0)

    nc.compile()
    return nc


_CACHE = {}


def _get_module(n_samp, experts=None):
    key = (n_samp, experts)
    if key not in _CACHE:
        _CACHE[key] = build_module(n_samp, experts=experts)
    return _CACHE[key]


def _routing_plan(treat, n_core):
    """Per-core stable sort by treatment so most blocks are single-expert.

    Schedule: pa pure-A blocks, 2 mixed blocks, rest pure-C.  Valid iff each
    core's t==0 count lands inside the mixed window — an ~11-sigma certainty
    for balanced random treatment; returns None otherwise (generic fallback).
    """
    nblk = n_core // SB
    if n_core % SB or nblk < 4:
        return None
    pa = nblk // 2 - 1
    lo, hi = pa * SB, (pa + 2) * SB
    perms = []
    for c in range(N_CORES):
        tc_ = treat[c * n_core:(c + 1) * n_core]
        c0 = int((tc_ == 0).sum())
        if not (lo <= c0 <= hi):
            return None
        perms.append(np.argsort(tc_, kind="stable") + c * n_core)
    experts = ("A",) * pa + ("both",) * 2 + ("C",) * (nblk - pa - 2)
    return np.concatenate(perms), experts


def kernel(**inputs):
    x = np.ascontiguousarray(np.asarray(inputs["input"], dtype=np.float32))
    treat = np.ascontiguousarray(np.asarray(inputs["treatment"], dtype=np.int32))
    n = x.shape[0]
    n_core = n // N_CORES

    plan = _routing_plan(treat, n_core)
    if plan is not None:
        perm, experts = plan
        x_k, t_k = x[perm], treat[perm]
    else:
        perm, experts = None, None
        x_k, t_k = x, treat

    nc = _get_module(n_core, experts)

    common = {}
    for name in WEIGHT_NAMES:
        common[name] = np.ascontiguousarray(np.asarray(inputs[name], np.float32))

    in_maps = []
    for c in range(N_CORES):
        sl = slice(c * n_core, (c + 1) * n_core)
        m = dict(common)
        m["xT"] = np.ascontiguousarray(x_k[sl].T)
        m["treat"] = np.ascontiguousarray(t_k[sl])
        in_maps.append(m)

    res = run_bass_kernel_spmd(nc, in_maps, core_ids=list(range(N_CORES)))
    y = np.concatenate([r["y"] for r in res.results])
    out = np.concatenate([np.ascontiguousarray(r["outT"].T)
                          for r in res.results], axis=0)
    if perm is not None:
        y_u = np.empty_like(y)
        out_u = np.empty_like(out)
        y_u[perm] = y
        out_u[perm] = out
        y, out = y_u, out_u
    return y, out
